# revision 16
# baseline (speedup 1.0000x reference)
"""Trainium2 Bass kernel for the Abbott STDP update (nn_Abbott_54889682042972).

Reference computation (per batch b, pre-synapse e, post-synapse o):
    I[b,o]            = sum_e Xpre[b,e,o] * W[b,e,o]
    dW_pot[b,e,o]     = Xpost[b,o] * xbar_pre[b,e,o] * A_p[e,o]
    dW_dep[b,e,o]     = Xpre[b,e,o] * xbar_post[b,o] * A_d[e,o]
    xbar_pre_new      = 0.95*xbar_pre + 0.05*Xpre
    xbar_post_new     = 0.9*xbar_post + 0.1*Xpost
    W_new             = clip(W + dW_pot - dW_dep, 0, 1)

Strategy (memory-bound problem; minimize HBM bytes and keep every DMA
contiguous):
  * Shard the post (o) axis across the 8 cores.  All contractions reduce
    only over e, so there are no collectives and A_p/A_d shard instead of
    replicate.
  * The host pre-transposes the big tensors to [b, o, e] layout so o sits
    on SBUF partitions and e on the free dim.  Xpost[b,o]/xbar_post[b,o]
    become per-partition scalars (fused scalar_tensor_tensor ops, no
    broadcast tiles), and the I-reduction runs along the free dim fused
    into the Xpre*W multiply via accum_out.
  * Xpre is binary (spikes), shipped as uint8 (4x fewer bytes) and cast
    to f32 on the otherwise-idle Activation engine.
  * Store-DMAs are issued from the Activation/GpSimd sequencers so their
    semaphore waits never head-of-line-block load issue on the SP
    sequencer.
"""

import sys

for _p in ("/opt/trn_rl_repo",):
    if _p not in sys.path:
        sys.path.insert(0, _p)

import numpy as np

B = 8
N = 2048
NCORES = 8
OS = N // NCORES  # o-slice per core = 256
OB = OS // 128  # o partition-blocks per core = 2
NCOL = OB * B  # 16 small-vector columns per core

ALPHA_P = 0.95
ALPHA_D = 0.9
WMAX = 1.0

IO_BUFS = 5
TMP_BUFS = 2
# The last TAIL_ITERS (b,ob) iterations are processed as TAIL_SPLIT sub-tiles
# along the free (e) dim so the pipeline tail drains faster (the final tile's
# cast->DVE->Pool->store chain is what keeps the DMA bus idle at the end).
TAIL_ITERS = 2
TAIL_SPLIT = 2

_PROGRAM_CACHE = {}


def _build_program():
    import concourse.bacc as bacc
    import concourse.mybir as mybir
    from concourse.tile import TileContext

    dt = mybir.dt.float32
    u8 = mybir.dt.uint8
    mult = mybir.AluOpType.mult
    add = mybir.AluOpType.add
    amax = mybir.AluOpType.max
    amin = mybir.AluOpType.min

    nc = bacc.Bacc("TRN2", debug=False, num_devices=NCORES)

    XpreT = nc.dram_tensor("XpreT", [B, OS, N], u8, kind="ExternalInput")
    WT = nc.dram_tensor("WT", [B, OS, N], dt, kind="ExternalInput")
    XbpT = nc.dram_tensor("XbpT", [B, OS, N], dt, kind="ExternalInput")
    ApT = nc.dram_tensor("ApT", [OS, N], dt, kind="ExternalInput")
    AdT = nc.dram_tensor("AdT", [OS, N], dt, kind="ExternalInput")
    XpostP = nc.dram_tensor("XpostP", [128, NCOL], dt, kind="ExternalInput")
    XbarPostP = nc.dram_tensor("XbarPostP", [128, NCOL], dt, kind="ExternalInput")

    WNewT = nc.dram_tensor("WNewT", [B, OS, N], dt, kind="ExternalOutput")
    XbpNewT = nc.dram_tensor("XbpNewT", [B, OS, N], dt, kind="ExternalOutput")
    IP = nc.dram_tensor("IP", [128, NCOL], dt, kind="ExternalOutput")
    XbarPostNewP = nc.dram_tensor(
        "XbarPostNewP", [128, NCOL], dt, kind="ExternalOutput"
    )

    def dma_store(out_ap, in_ap):
        # Activation-engine sequencer: store waits must not block SP load issue
        nc.scalar.dma_start(out=out_ap, in_=in_ap)

    with TileContext(nc) as tc:
        with (
            tc.tile_pool(name="const", bufs=1) as cpool,
            tc.tile_pool(name="io", bufs=IO_BUFS) as io,
            tc.tile_pool(name="tmp", bufs=TMP_BUFS) as tmp,
            tc.tile_pool(name="scr", bufs=1) as scr,
        ):
            ap_t, ad_t = [], []
            for ob in range(OB):
                t = cpool.tile([128, N], dt, tag=f"ap{ob}")
                nc.sync.dma_start(out=t, in_=ApT[ob * 128 : (ob + 1) * 128, :])
                ap_t.append(t)
                t = cpool.tile([128, N], dt, tag=f"ad{ob}")
                nc.sync.dma_start(out=t, in_=AdT[ob * 128 : (ob + 1) * 128, :])
                ad_t.append(t)

            xpost_sc = cpool.tile([128, NCOL], dt, tag="xpost_sc")
            nc.sync.dma_start(out=xpost_sc, in_=XpostP[:, :])
            xbpost_sc = cpool.tile([128, NCOL], dt, tag="xbpost_sc")
            nc.sync.dma_start(out=xbpost_sc, in_=XbarPostP[:, :])

            ip_t = cpool.tile([128, NCOL], dt, tag="ip")

            # xbar_post_new = (Xpost - xbar_post)*(1-alpha_d) + xbar_post
            xpn_t = cpool.tile([128, NCOL], dt, tag="xpn")
            nc.vector.tensor_sub(xpn_t, xpost_sc, xbpost_sc)
            nc.vector.scalar_tensor_tensor(
                xpn_t, xpn_t, 1.0 - ALPHA_D, xbpost_sc, mult, add
            )
            nc.sync.dma_start(out=XbarPostNewP[:, :], in_=xpn_t)

            for b in range(B):
                for ob in range(OB):
                    col = ob * B + b
                    osl = slice(ob * 128, (ob + 1) * 128)
                    iter_idx = b * OB + ob
                    n_ft = TAIL_SPLIT if iter_idx >= B * OB - TAIL_ITERS else 1
                    FT = N // n_ft
                    if n_ft > 1:
                        iacc_t = tmp.tile([128, n_ft], dt, tag="iacc")
                    else:
                        iacc_t = None

                    for ft in range(n_ft):
                        fsl = slice(ft * FT, (ft + 1) * FT)
                        xpre8_t = io.tile([128, FT], u8, tag="xpre8")
                        nc.sync.dma_start(out=xpre8_t, in_=XpreT[b, osl, fsl])
                        w_t = io.tile([128, FT], dt, tag="w")
                        nc.sync.dma_start(out=w_t, in_=WT[b, osl, fsl])
                        xbp_t = io.tile([128, FT], dt, tag="xbp")
                        nc.sync.dma_start(out=xbp_t, in_=XbpT[b, osl, fsl])

                        # u8 -> f32 spike tile on the otherwise-idle ACT engine
                        xpre_t = io.tile([128, FT], dt, tag="xpre")
                        nc.scalar.copy(xpre_t, xpre8_t)

                        scratch = scr.tile([128, FT], dt, tag="scratch")
                        t_a = tmp.tile([128, FT], dt, tag="ta")
                        t_d = tmp.tile([128, FT], dt, tag="td")

                        xp_s = xpost_sc[:, col : col + 1]
                        xbp_s = xbpost_sc[:, col : col + 1]
                        acc_slot = (
                            ip_t[:, col : col + 1]
                            if n_ft == 1
                            else iacc_t[:, ft : ft + 1]
                        )

                        # I[col] = sum_e Xpre*W (fused multiply + free-dim reduce)
                        nc.vector.scalar_tensor_tensor(
                            scratch,
                            xpre_t,
                            1.0,
                            w_t,
                            mult,
                            mult,
                            accum_out=acc_slot,
                        )
                        # dW_pot = (A_p * Xpost) * xbar_pre
                        nc.vector.scalar_tensor_tensor(
                            t_a, ap_t[ob][:, fsl], xp_s, xbp_t, mult, mult
                        )
                        # dW_dep = (A_d * xbar_post) * Xpre
                        nc.vector.scalar_tensor_tensor(
                            t_d, ad_t[ob][:, fsl], xbp_s, xpre_t, mult, mult
                        )
                        # W_new = clip(W + dW_pot - dW_dep, 0, 1)   (GpSimd;
                        # scalar_tensor_tensor is DVE-only in this walrus, so
                        # Pool gets the plain tensor_tensor/tensor_scalar chain)
                        nc.gpsimd.tensor_add(w_t, w_t, t_a)
                        nc.gpsimd.tensor_sub(w_t, w_t, t_d)
                        nc.gpsimd.tensor_scalar(w_t, w_t, 0.0, 1.0, amax, amin)
                        dma_store(WNewT[b, osl, fsl], w_t)
                        # xbar_pre_new = (Xpre - xbar_pre)*(1-alpha_p) + xbar_pre
                        nc.vector.tensor_sub(xpre_t, xpre_t, xbp_t)
                        nc.vector.scalar_tensor_tensor(
                            xpre_t, xpre_t, 1.0 - ALPHA_P, xbp_t, mult, add
                        )
                        dma_store(XbpNewT[b, osl, fsl], xpre_t)

                    if n_ft == 2:
                        nc.vector.tensor_add(
                            ip_t[:, col : col + 1], iacc_t[:, 0:1], iacc_t[:, 1:2]
                        )

            nc.sync.dma_start(out=IP[:, :], in_=ip_t)

    nc.compile()
    return nc


def _get_program():
    if "nc" not in _PROGRAM_CACHE:
        _PROGRAM_CACHE["nc"] = _build_program()
    return _PROGRAM_CACHE["nc"]


def _pack_small(v):
    # [B, OS] -> [128, NCOL] with column index ob*B + b
    return np.ascontiguousarray(
        v.reshape(B, OB, 128).transpose(2, 1, 0).reshape(128, NCOL)
    )


def _unpack_small(vp):
    # [128, NCOL] -> [B, OS]
    return np.ascontiguousarray(
        vp.reshape(128, OB, B).transpose(2, 1, 0).reshape(B, OS)
    )


def make_in_maps(Xpre, Xpost, W, xbar_pre, xbar_post, A_p, A_d):
    from concurrent.futures import ThreadPoolExecutor

    def one_core(c):
        osl = slice(c * OS, (c + 1) * OS)
        return {
            "XpreT": np.ascontiguousarray(
                Xpre[:, :, osl].transpose(0, 2, 1)
            ).astype(np.uint8),
            "WT": np.ascontiguousarray(W[:, :, osl].transpose(0, 2, 1)),
            "XbpT": np.ascontiguousarray(xbar_pre[:, :, osl].transpose(0, 2, 1)),
            "ApT": np.ascontiguousarray(A_p[:, osl].T),
            "AdT": np.ascontiguousarray(A_d[:, osl].T),
            "XpostP": _pack_small(Xpost[:, osl]),
            "XbarPostP": _pack_small(xbar_post[:, osl]),
        }

    with ThreadPoolExecutor(max_workers=NCORES) as ex:
        return list(ex.map(one_core, range(NCORES)))


def kernel(Xpre, Xpost, W, xbar_pre, xbar_post, A_p, A_d, **_ignored):
    from concourse.bass_utils import run_bass_kernel_spmd

    Xpre = np.asarray(Xpre, dtype=np.float32)
    Xpost = np.asarray(Xpost, dtype=np.float32)
    W = np.asarray(W, dtype=np.float32)
    xbar_pre = np.asarray(xbar_pre, dtype=np.float32)
    xbar_post = np.asarray(xbar_post, dtype=np.float32)
    A_p = np.asarray(A_p, dtype=np.float32)
    A_d = np.asarray(A_d, dtype=np.float32)

    in_maps = make_in_maps(Xpre, Xpost, W, xbar_pre, xbar_post, A_p, A_d)

    import time

    nc = _get_program()
    results = None
    last_exc = None
    for _attempt in range(3):
        try:
            res = run_bass_kernel_spmd(nc, in_maps, core_ids=list(range(NCORES)))
            results = res.results
            break
        except Exception as exc:  # transient device wedges have been observed
            last_exc = exc
            time.sleep(5.0)  # give a wedged exec unit time to recover
    if results is None:
        raise last_exc

    I = np.empty((B, N), dtype=np.float32)
    xbar_pre_new = np.empty((B, N, N), dtype=np.float32)
    xbar_post_new = np.empty((B, N), dtype=np.float32)
    W_new = np.empty((B, N, N), dtype=np.float32)

    from concurrent.futures import ThreadPoolExecutor

    def gather_core(c):
        osl = slice(c * OS, (c + 1) * OS)
        r = results[c]
        I[:, osl] = _unpack_small(r["IP"])
        xbar_post_new[:, osl] = _unpack_small(r["XbarPostNewP"])
        W_new[:, :, osl] = r["WNewT"].transpose(0, 2, 1)
        xbar_pre_new[:, :, osl] = r["XbpNewT"].transpose(0, 2, 1)

    with ThreadPoolExecutor(max_workers=NCORES) as ex:
        list(ex.map(gather_core, range(NCORES)))

    return I, xbar_pre_new, xbar_post_new, W_new
